# revision 7
# baseline (speedup 1.0000x reference)
"""Trainium2 Bass kernel for nn_MultiHeadAttention_85229331022244.

Computation (per batch b):
  xh = x.reshape(B,T,64,16); q/k/v = per-head 64x64 projections of xh
  q,k: interleaved RoPE over the FULL 1024-dim feature axis
  scores = q @ k.T / sqrt(1024)  (single attention map over full D)
  causal softmax; y = attn @ v

Sharding: core b -> batch b (4 cores used).  One core owns the whole
batch, so x is shipped to the device exactly once and K/Q/V projections
share the same x tiles.  Host<->device traffic is the metric driver
(memory regime): RoPE cos/sin tables are reconstructed on-device from
factorized half-tables (angle addition), causal masks are generated
on-device with affine_select, and the output returns as bf16.

Device layout trick: heads are reordered even-first and paired so the
projections become 8 block-diagonal 128x128 matmuls that produce
K^T/Q^T directly in [feature-on-partition, token] layout, with RoPE
partner features living in chunk c and c+4 at the same partition index.
"""

import math
from contextlib import ExitStack

import numpy as np
import ml_dtypes

import concourse.bass as bass
import concourse.mybir as mybir
import concourse.tile as tile
from concourse import bacc
from concourse.bass import ts, ds
from concourse.masks import make_identity

BF16 = ml_dtypes.bfloat16

D_MODEL = 1024
N_HEADS = 16
HEAD_D = 64
ROPE_BASE = 10000.0
GAMMA = 1.0 / math.sqrt(D_MODEL)
T = 4096
NT = 32  # 128-row query tiles per batch
NS = 8   # 512-token stripes

# head pairs per 128-row chunk; chunks 0-3 = even heads, 4-7 = odd heads
HEAD_PAIRS = [(0, 2), (4, 6), (8, 10), (12, 14), (1, 3), (5, 7), (9, 11), (13, 15)]


def _feature_perm():
    """perm[c*128 + p] = original feature index for kernel row (c, p)."""
    perm = np.zeros(1024, dtype=np.int64)
    for c, (ha, hb) in enumerate(HEAD_PAIRS):
        for p in range(128):
            h = ha if p < 64 else hb
            perm[c * 128 + p] = (p % 64) * 16 + h
    return perm


PERM = _feature_perm()
INV_PERM = np.argsort(PERM)


def _block_weights(w):
    """w: (64, 64, 16) -> (8, 128, 128) block-diag per chunk, bf16."""
    out = np.zeros((8, 128, 128), dtype=np.float32)
    for c, (ha, hb) in enumerate(HEAD_PAIRS):
        out[c, :64, :64] = w[:, :, ha]
        out[c, 64:, 64:] = w[:, :, hb]
    return out.astype(BF16)


def _inv_freq():
    """[4, 128] rope inverse frequencies for chunks 0-3 (partners 4-7)."""
    p = np.arange(128)
    out = np.zeros((4, 128), dtype=np.float64)
    for c in range(4):
        f = (p % 64) * 8 + (2 * c + p // 64)
        out[c] = ROPE_BASE ** (-f.astype(np.float64) / 512.0)
    return out


def _rope_factor_tables():
    """Angle-addition factorization: ang(p, 512*s + u) = hi(p,s) + lo(p,u).

    Returns cosL/sinL [4,128,512] and cosH/sinH [4,128,NS] in bf16.
    """
    invf = _inv_freq()  # [4, 128]
    u = np.arange(512, dtype=np.float64)
    s = np.arange(NS, dtype=np.float64) * 512.0
    lo = invf[:, :, None] * u[None, None, :]   # [4,128,512]
    hi = invf[:, :, None] * s[None, None, :]   # [4,128,NS]
    return (
        np.cos(lo).astype(BF16),
        np.sin(lo).astype(BF16),
        np.cos(hi).astype(np.float32),
        np.sin(hi).astype(np.float32),
    )


def build_nc():
    """Build the (identical-on-all-cores) Bass program for one full batch."""
    dt = mybir.dt
    nc = bacc.Bacc("TRN2", target_bir_lowering=False)
    xpT = nc.dram_tensor("xpT", [8, 128, T], dt.bfloat16, kind="ExternalInput")
    w2q = nc.dram_tensor("w2q", [8, 128, 128], dt.bfloat16, kind="ExternalInput")
    w2k = nc.dram_tensor("w2k", [8, 128, 128], dt.bfloat16, kind="ExternalInput")
    w2v = nc.dram_tensor("w2v", [8, 128, 128], dt.bfloat16, kind="ExternalInput")
    cosL = nc.dram_tensor("cosL", [4, 128, 512], dt.bfloat16, kind="ExternalInput")
    sinL = nc.dram_tensor("sinL", [4, 128, 512], dt.bfloat16, kind="ExternalInput")
    cosH = nc.dram_tensor("cosH", [4, 128, NS], dt.float32, kind="ExternalInput")
    sinH = nc.dram_tensor("sinH", [4, 128, NS], dt.float32, kind="ExternalInput")
    y = nc.dram_tensor("y", [T, 1024], dt.bfloat16, kind="ExternalOutput")

    with tile.TileContext(nc) as tc, ExitStack() as ctx:
        const = ctx.enter_context(tc.tile_pool(name="const", bufs=1))
        kv = ctx.enter_context(tc.tile_pool(name="kv", bufs=1))
        qpool = ctx.enter_context(tc.tile_pool(name="qpool", bufs=2))
        xpool = ctx.enter_context(tc.tile_pool(name="xpool", bufs=2))
        cspool = ctx.enter_context(tc.tile_pool(name="cspool", bufs=2))
        rtmp = ctx.enter_context(tc.tile_pool(name="rtmp", bufs=2))
        ppool = ctx.enter_context(tc.tile_pool(name="ppool", bufs=2))
        ptpool = ctx.enter_context(tc.tile_pool(name="ptpool", bufs=2))
        ypool = ctx.enter_context(tc.tile_pool(name="ypool", bufs=2))
        lpool = ctx.enter_context(tc.tile_pool(name="lpool", bufs=2))
        psum = ctx.enter_context(tc.tile_pool(name="psum", bufs=2, space="PSUM"))
        # YL/YH double-buffered; V-projection PSUM shares the same slots
        # (proj and attention never need them simultaneously beyond the
        # rotation the scheduler already enforces).
        psum1 = ctx.enter_context(tc.tile_pool(name="psum1", bufs=2, space="PSUM"))

        # constants
        ident = const.tile([128, 128], dt.bfloat16, tag="ident", name="ident")
        make_identity(nc, ident)
        wq_sb, wk_sb, wv_sb = [], [], []
        for c in range(8):
            for name, dram, lst in (
                ("wq", w2q, wq_sb),
                ("wk", w2k, wk_sb),
                ("wv", w2v, wv_sb),
            ):
                wt = const.tile([128, 128], dt.bfloat16, tag=f"{name}{c}", name=f"{name}{c}")
                nc.sync.dma_start(wt[:], dram[c])
                lst.append(wt)
        cosL_sb, sinL_sb, cosH_sb, sinH_sb = [], [], [], []
        for c in range(4):
            for name, dram, lst, w_, dt_ in (
                ("cl", cosL, cosL_sb, 512, dt.bfloat16),
                ("sl", sinL, sinL_sb, 512, dt.bfloat16),
                ("ch", cosH, cosH_sb, NS, dt.float32),
                ("sh", sinH, sinH_sb, NS, dt.float32),
            ):
                t_ = const.tile([128, w_], dt_, tag=f"{name}{c}", name=f"{name}{c}")
                nc.sync.dma_start(t_[:], dram[c])
                lst.append(t_)
        # causal masks for the last stripe of each q-tile: pattern depends only
        # on r = G mod 4.  mask_r[p, c] = 0 if c <= 128*r + p else -1e9.
        masks = []
        for r in range(4):
            mt = const.tile([128, 512], dt.float32, tag=f"mask{r}", name=f"mask{r}")
            nc.gpsimd.memset(mt[:], 0.0)
            nc.gpsimd.affine_select(
                out=mt[:],
                in_=mt[:],
                compare_op=mybir.AluOpType.is_ge,
                fill=-1e9,
                base=r * 128,
                pattern=[[-1, 512]],
                channel_multiplier=1,
            )
            masks.append(mt)

        # resident K^T (per chunk c and 512-key stripe s) and V (per-stripe)
        KT = {}
        for s in range(NS):
            for c in range(8):
                KT[(c, s)] = kv.tile([128, 512], dt.bfloat16, tag=f"kt{c}_{s}",
                                     name=f"kt{c}_{s}")
        V = [
            kv.tile([128, 4, 1024], dt.bfloat16, tag=f"v{s}", name=f"v{s}")
            for s in range(NS)
        ]
        QT = {}  # streamed, tags per chunk

        def emit_proj_stripe(s):
            """K^T, Q^T, V for the 512-token stripe s (shared x / cos / sin)."""
            sl = ds(s * 512, 512)
            for cp in range(4):
                xa = xpool.tile([128, 512], dt.bfloat16, tag="xa", name="xa")
                xb = xpool.tile([128, 512], dt.bfloat16, tag="xb", name="xb")
                nc.sync.dma_start(xa[:], xpT[cp, :, sl])
                nc.sync.dma_start(xb[:], xpT[cp + 4, :, sl])
                # reconstruct cos/sin for (chunk cp, stripe s) via angle addition
                cos = cspool.tile([128, 512], dt.bfloat16, tag="cos", name="cos")
                sin = cspool.tile([128, 512], dt.bfloat16, tag="sin", name="sin")
                t1 = cspool.tile([128, 512], dt.bfloat16, tag="t1", name="t1")
                t2 = cspool.tile([128, 512], dt.bfloat16, tag="t2", name="t2")
                chs = cosH_sb[cp][:, ds(s, 1)]
                shs = sinH_sb[cp][:, ds(s, 1)]
                nc.vector.tensor_scalar_mul(t1[:], cosL_sb[cp][:], chs)
                nc.vector.tensor_scalar_mul(t2[:], sinL_sb[cp][:], shs)
                nc.vector.tensor_sub(cos[:], t1[:], t2[:])
                t3 = cspool.tile([128, 512], dt.bfloat16, tag="t1", name="t3")
                t4 = cspool.tile([128, 512], dt.bfloat16, tag="t2", name="t4")
                nc.vector.tensor_scalar_mul(t3[:], cosL_sb[cp][:], shs)
                nc.vector.tensor_scalar_mul(t4[:], sinL_sb[cp][:], chs)
                nc.vector.tensor_add(sin[:], t3[:], t4[:])

                def rope_pair(w_sb, out_e, out_o):
                    pe = psum.tile([128, 512], dt.float32, tag="A", name="A")
                    po = psum.tile([128, 512], dt.float32, tag="B", name="B")
                    nc.tensor.matmul(pe[:], lhsT=w_sb[cp][:], rhs=xa[:],
                                     start=True, stop=True)
                    nc.tensor.matmul(po[:], lhsT=w_sb[cp + 4][:], rhs=xb[:],
                                     start=True, stop=True)
                    ke = rtmp.tile([128, 512], dt.bfloat16, tag="ke", name="ke")
                    ko = rtmp.tile([128, 512], dt.bfloat16, tag="ko", name="ko")
                    nc.scalar.copy(ke[:], pe[:])
                    nc.scalar.copy(ko[:], po[:])
                    ta = rtmp.tile([128, 512], dt.bfloat16, tag="ta", name="ta")
                    tb = rtmp.tile([128, 512], dt.bfloat16, tag="tb", name="tb")
                    nc.vector.tensor_mul(ta[:], ke[:], cos[:])
                    nc.vector.tensor_mul(tb[:], ko[:], sin[:])
                    nc.vector.tensor_sub(out_e[:], ta[:], tb[:])
                    ta2 = rtmp.tile([128, 512], dt.bfloat16, tag="ta", name="ta")
                    tb2 = rtmp.tile([128, 512], dt.bfloat16, tag="tb", name="tb")
                    nc.vector.tensor_mul(ta2[:], ke[:], sin[:])
                    nc.vector.tensor_mul(tb2[:], ko[:], cos[:])
                    nc.vector.tensor_add(out_o[:], ta2[:], tb2[:])

                rope_pair(wk_sb, KT[(cp, s)], KT[(cp + 4, s)])
                QT[(cp, s)] = qpool.tile([128, 512], dt.bfloat16, tag=f"qt{cp}",
                                         name=f"qt{cp}")
                QT[(cp + 4, s)] = qpool.tile([128, 512], dt.bfloat16,
                                             tag=f"qt{cp + 4}", name=f"qt{cp + 4}")
                rope_pair(wq_sb, QT[(cp, s)], QT[(cp + 4, s)])

                va = psum1.tile([128, 4, 128], dt.float32, tag="YL", name="VA")
                vb = psum1.tile([128, 4, 128], dt.float32, tag="YH", name="VB")
                for sub in range(4):
                    nc.tensor.matmul(
                        va[:, sub, :], lhsT=xa[:, ts(sub, 128)], rhs=wv_sb[cp][:],
                        start=True, stop=True,
                    )
                    nc.tensor.matmul(
                        vb[:, sub, :], lhsT=xb[:, ts(sub, 128)], rhs=wv_sb[cp + 4][:],
                        start=True, stop=True,
                    )
                nc.any.tensor_copy(V[s][:, :, ds(cp * 128, 128)], va[:])
                nc.any.tensor_copy(V[s][:, :, ds((cp + 4) * 128, 128)], vb[:])

        def emit_q_tile(G):
            nblk = G + 1
            nst = (nblk + 3) // 4
            wlast = (nblk - 4 * (nst - 1)) * 128
            qs, qoff = G // 4, (G % 4) * 128
            y_lo = psum1.tile([128, 512], dt.float32, tag="YL", name="YL")
            y_hi = psum1.tile([128, 512], dt.float32, tag="YH", name="YH")
            l_parts = lpool.tile([128, NS], dt.float32, tag="lp", name="lp")
            for t in range(nst):
                w = 512 if t < nst - 1 else wlast
                S = psum.tile([128, 512], dt.float32, tag="A", name="A")
                for c in range(8):
                    nc.tensor.matmul(
                        S[:, :w],
                        lhsT=QT[(c, qs)][:, ds(qoff, 128)],
                        rhs=KT[(c, t)][:, :w],
                        start=(c == 0), stop=(c == 7),
                    )
                if t == nst - 1:
                    nc.vector.tensor_add(S[:, :w], S[:, :w], masks[G % 4][:, :w])
                P = ppool.tile([128, 512], dt.bfloat16, tag="p", name="p")
                nc.scalar.activation(
                    P[:, :w], S[:, :w], mybir.ActivationFunctionType.Exp,
                    scale=GAMMA, accum_out=l_parts[:, ds(t, 1)],
                )
                nb = w // 128
                pt_ps = psum.tile([128, 512], dt.bfloat16, tag="B", name="B")
                for b in range(nb):
                    nc.tensor.transpose(pt_ps[:, ts(b, 128)], P[:, ts(b, 128)],
                                        ident[:])
                pt = ptpool.tile([128, 512], dt.bfloat16, tag="pt", name="pt")
                nc.scalar.copy(pt[:, :w], pt_ps[:, :w])
                for b in range(nb):
                    blk = t * 4 + b
                    vs = V[blk // 4]
                    nc.tensor.matmul(y_lo[:], lhsT=pt[:, ts(b, 128)],
                                     rhs=vs[:, blk % 4, 0:512],
                                     start=(blk == 0), stop=(blk == nblk - 1))
                    nc.tensor.matmul(y_hi[:], lhsT=pt[:, ts(b, 128)],
                                     rhs=vs[:, blk % 4, 512:1024],
                                     start=(blk == 0), stop=(blk == nblk - 1))
            lsum = lpool.tile([128, 1], dt.float32, tag="ls", name="ls")
            linv = lpool.tile([128, 1], dt.float32, tag="li", name="li")
            nc.vector.tensor_reduce(lsum[:], l_parts[:, :nst],
                                    mybir.AxisListType.X, mybir.AluOpType.add)
            nc.vector.reciprocal(linv[:], lsum[:])
            y_sb = ypool.tile([128, 1024], dt.bfloat16, tag="y", name="y")
            nc.vector.tensor_scalar_mul(y_sb[:, 0:512], y_lo[:], linv[:])
            nc.vector.tensor_scalar_mul(y_sb[:, 512:1024], y_hi[:], linv[:])
            nc.sync.dma_start(y[ts(G, 128), :], y_sb[:])

        for s in range(NS):
            emit_proj_stripe(s)
            for G in range(4 * s, 4 * s + 4):
                emit_q_tile(G)

    nc.compile()
    return nc


# ------------------------- host side -------------------------


def prep_core_inputs(xb, w2q, w2k, w2v, tables):
    """Inputs for one core: batch slice xb (T, 1024) fp32."""
    cosL_t, sinL_t, cosH_t, sinH_t = tables
    xpT = np.ascontiguousarray(xb.T[PERM].reshape(8, 128, T)).astype(BF16)
    return {
        "xpT": xpT,
        "w2q": w2q,
        "w2k": w2k,
        "w2v": w2v,
        "cosL": cosL_t,
        "sinL": sinL_t,
        "cosH": cosH_t,
        "sinH": sinH_t,
    }


def core_model(inp):
    """Numpy model of one core's program (fp32 math, for tests)."""
    xpT = inp["xpT"].astype(np.float32)
    w2q = inp["w2q"].astype(np.float32)
    w2k = inp["w2k"].astype(np.float32)
    w2v = inp["w2v"].astype(np.float32)
    cosL_t = inp["cosL"].astype(np.float32)
    sinL_t = inp["sinL"].astype(np.float32)
    cosH_t = inp["cosH"].astype(np.float32)
    sinH_t = inp["sinH"].astype(np.float32)
    # reconstruct cos/sin as the device does
    cos = np.zeros((4, 128, T), dtype=np.float32)
    sin = np.zeros((4, 128, T), dtype=np.float32)
    for s in range(NS):
        ch = cosH_t[:, :, s:s + 1]
        sh = sinH_t[:, :, s:s + 1]
        cos[:, :, s * 512:(s + 1) * 512] = cosL_t * ch - sinL_t * sh
        sin[:, :, s * 512:(s + 1) * 512] = cosL_t * sh + sinL_t * ch

    def proj_T(w2):  # -> [8, 128, T]
        return np.stack([w2[c].T @ xpT[c] for c in range(8)])

    def rope(zT):
        out = np.empty_like(zT)
        for c in range(4):
            e, o = zT[c], zT[c + 4]
            out[c] = e * cos[c] - o * sin[c]
            out[c + 4] = e * sin[c] + o * cos[c]
        return out

    kT = rope(proj_T(w2k)).reshape(1024, T)
    qT = rope(proj_T(w2q)).reshape(1024, T)
    v = np.concatenate([w2v[c].T @ xpT[c] for c in range(8)], axis=0).T  # [T,1024]

    yout = np.zeros((T, 1024), dtype=np.float32)
    for G in range(NT):
        nblk = G + 1
        q = qT[:, G * 128:(G + 1) * 128].T  # [128, 1024]
        keys = kT[:, :nblk * 128]
        S = q @ keys
        k0 = 4 * ((nblk + 3) // 4 - 1) * 128
        r = G % 4
        pcol = np.arange(128)[:, None]
        ccol = np.arange(nblk * 128 - k0)[None, :]
        S[:, k0:] += np.where(ccol <= r * 128 + pcol, 0.0, -1e9)
        P = np.exp(GAMMA * S)
        yout[G * 128:(G + 1) * 128] = (P @ v[:nblk * 128]) / P.sum(1, keepdims=True)
    return yout


_NC_CACHE = {}
last_in_maps = None


def kernel(x, w_q, w_k, w_v):
    global last_in_maps
    from concourse.bass_utils import run_bass_kernel_spmd

    B, Tx, D = x.shape
    assert (B, Tx, D) == (4, 4096, 1024)
    x = np.asarray(x, dtype=np.float32)
    w2q = _block_weights(np.asarray(w_q, dtype=np.float32))
    w2k = _block_weights(np.asarray(w_k, dtype=np.float32))
    w2v = _block_weights(np.asarray(w_v, dtype=np.float32))
    tables = _rope_factor_tables()

    in_maps = [prep_core_inputs(x[b], w2q, w2k, w2v, tables) for b in range(4)]
    last_in_maps = in_maps

    if "nc" not in _NC_CACHE:
        _NC_CACHE["nc"] = build_nc()
    nc = _NC_CACHE["nc"]

    res = run_bass_kernel_spmd(nc, in_maps, core_ids=list(range(4)))
    out = np.zeros((B, Tx, D), dtype=np.float32)
    for b in range(4):
        out[b] = res.results[b]["y"].astype(np.float32)[:, INV_PERM]
    return out


# revision 16
# speedup vs baseline: 1.0950x; 1.0950x over previous
"""Trainium2 Bass kernel for nn_MultiHeadAttention_85229331022244.

Computation (per batch b):
  xh = x.reshape(B,T,64,16); q/k/v = per-head 64x64 projections of xh
  q,k: interleaved RoPE over the FULL 1024-dim feature axis
  scores = q @ k.T / sqrt(1024)  (single attention map over full D)
  causal softmax; y = attn @ v

Sharding: core b -> batch b (4 cores used).  One core owns the whole
batch, so x is shipped to the device exactly once and K/Q/V projections
share the same x tiles.  Host<->device traffic is the metric driver
(memory regime): RoPE cos/sin tables are reconstructed on-device from
factorized half-tables (angle addition), causal masks are generated
on-device with affine_select, and the output returns as bf16.

Device layout trick: heads are reordered even-first and paired so the
projections become 8 block-diagonal 128x128 matmuls that produce
K^T/Q^T directly in [feature-on-partition, token] layout, with RoPE
partner features living in chunk c and c+4 at the same partition index.
"""

import math
from contextlib import ExitStack

import numpy as np
import ml_dtypes

import concourse.bass as bass
import concourse.mybir as mybir
import concourse.tile as tile
from concourse import bacc
from concourse.bass import ts, ds
from concourse.masks import make_identity

BF16 = ml_dtypes.bfloat16

D_MODEL = 1024
N_HEADS = 16
HEAD_D = 64
ROPE_BASE = 10000.0
GAMMA = 1.0 / math.sqrt(D_MODEL)
T = 4096
NT = 32  # 128-row query tiles per batch
NS = 8   # 512-token stripes

# head pairs per 128-row chunk; chunks 0-3 = even heads, 4-7 = odd heads
HEAD_PAIRS = [(0, 2), (4, 6), (8, 10), (12, 14), (1, 3), (5, 7), (9, 11), (13, 15)]


def _feature_perm():
    """perm[c*128 + p] = original feature index for kernel row (c, p)."""
    perm = np.zeros(1024, dtype=np.int64)
    for c, (ha, hb) in enumerate(HEAD_PAIRS):
        for p in range(128):
            h = ha if p < 64 else hb
            perm[c * 128 + p] = (p % 64) * 16 + h
    return perm


PERM = _feature_perm()
INV_PERM = np.argsort(PERM)


def _block_weights(w):
    """w: (64, 64, 16) -> (8, 128, 128) block-diag per chunk, bf16."""
    out = np.zeros((8, 128, 128), dtype=np.float32)
    for c, (ha, hb) in enumerate(HEAD_PAIRS):
        out[c, :64, :64] = w[:, :, ha]
        out[c, 64:, 64:] = w[:, :, hb]
    return out.astype(BF16)


def _inv_freq():
    """[4, 128] rope inverse frequencies for chunks 0-3 (partners 4-7)."""
    p = np.arange(128)
    out = np.zeros((4, 128), dtype=np.float64)
    for c in range(4):
        f = (p % 64) * 8 + (2 * c + p // 64)
        out[c] = ROPE_BASE ** (-f.astype(np.float64) / 512.0)
    return out


def _rope_factor_tables():
    """Angle-addition factorization: ang(p, 512*s + u) = hi(p,s) + lo(p,u).

    Returns cosL/sinL [4,128,512] and cosH/sinH [4,128,NS] in bf16.
    """
    invf = _inv_freq()  # [4, 128]
    u = np.arange(512, dtype=np.float64)
    s = np.arange(NS, dtype=np.float64) * 512.0
    lo = invf[:, :, None] * u[None, None, :]   # [4,128,512]
    hi = invf[:, :, None] * s[None, None, :]   # [4,128,NS]
    return (
        np.cos(lo).astype(BF16),
        np.sin(lo).astype(BF16),
        np.cos(hi).astype(np.float32),
        np.sin(hi).astype(np.float32),
    )


def build_nc():
    """Build the (identical-on-all-cores) Bass program for one full batch."""
    dt = mybir.dt
    nc = bacc.Bacc("TRN2", target_bir_lowering=False)
    xpT = nc.dram_tensor("xpT", [8, 128, T], dt.bfloat16, kind="ExternalInput")
    # all bf16 constants packed per-partition: wq|wk|wv (3x1024) then
    # cosL|sinL (2x2048) -> one DMA instead of ~40
    wtab = nc.dram_tensor("wtab", [128, 7168], dt.bfloat16, kind="ExternalInput")
    # fp32 per-partition rope "high" scalars: cosH (4x8) | sinH (4x8)
    httab = nc.dram_tensor("httab", [128, 64], dt.float32, kind="ExternalInput")
    y = nc.dram_tensor("y", [T, 1024], dt.bfloat16, kind="ExternalOutput")

    with tile.TileContext(nc) as tc, ExitStack() as ctx:
        const = ctx.enter_context(tc.tile_pool(name="const", bufs=1))
        kv = ctx.enter_context(tc.tile_pool(name="kv", bufs=1))
        qpool = ctx.enter_context(tc.tile_pool(name="qpool", bufs=2))
        xpool = ctx.enter_context(tc.tile_pool(name="xpool", bufs=2))
        cspool = ctx.enter_context(tc.tile_pool(name="cspool", bufs=2))
        rtmp = ctx.enter_context(tc.tile_pool(name="rtmp", bufs=2))
        ppool = ctx.enter_context(tc.tile_pool(name="ppool", bufs=2))
        ptpool = ctx.enter_context(tc.tile_pool(name="ptpool", bufs=2))
        ypool = ctx.enter_context(tc.tile_pool(name="ypool", bufs=2))
        lpool = ctx.enter_context(tc.tile_pool(name="lpool", bufs=2))
        psum = ctx.enter_context(tc.tile_pool(name="psum", bufs=2, space="PSUM"))
        # YL/YH double-buffered; V-projection PSUM shares the same slots
        # (proj and attention never need them simultaneously beyond the
        # rotation the scheduler already enforces).
        psum1 = ctx.enter_context(tc.tile_pool(name="psum1", bufs=2, space="PSUM"))

        # constants
        ident = const.tile([128, 128], dt.bfloat16, tag="ident", name="ident")
        make_identity(nc, ident)
        wtab_sb = const.tile([128, 7168], dt.bfloat16, tag="wtab", name="wtab")
        nc.sync.dma_start(wtab_sb[:], wtab[:])
        httab_sb = const.tile([128, 64], dt.float32, tag="httab", name="httab")
        nc.sync.dma_start(httab_sb[:], httab[:])
        wq_sb = [wtab_sb[:, ds(c * 128, 128)] for c in range(8)]
        wk_sb = [wtab_sb[:, ds(1024 + c * 128, 128)] for c in range(8)]
        wv_sb = [wtab_sb[:, ds(2048 + c * 128, 128)] for c in range(8)]
        cosL_sb = [wtab_sb[:, ds(3072 + c * 512, 512)] for c in range(4)]
        sinL_sb = [wtab_sb[:, ds(5120 + c * 512, 512)] for c in range(4)]
        cosH_sb = [httab_sb[:, ds(c * NS, NS)] for c in range(4)]
        sinH_sb = [httab_sb[:, ds(32 + c * NS, NS)] for c in range(4)]
        # causal masks for the last stripe of each q-tile: pattern depends only
        # on r = G mod 4.  mask_r[p, c] = 0 if c <= 128*r + p else -1e9.
        masks = []
        for r in range(4):
            mt = const.tile([128, 512], dt.float32, tag=f"mask{r}", name=f"mask{r}")
            nc.gpsimd.memset(mt[:], 0.0)
            nc.gpsimd.affine_select(
                out=mt[:],
                in_=mt[:],
                compare_op=mybir.AluOpType.is_ge,
                fill=-1e9,
                base=r * 128,
                pattern=[[-1, 512]],
                channel_multiplier=1,
            )
            masks.append(mt)

        # resident K^T (per chunk c and 512-key stripe s) and V (per-stripe)
        KT = {}
        for s in range(NS):
            for c in range(8):
                KT[(c, s)] = kv.tile([128, 512], dt.bfloat16, tag=f"kt{c}_{s}",
                                     name=f"kt{c}_{s}")
        V = [
            kv.tile([128, 4, 1024], dt.bfloat16, tag=f"v{s}", name=f"v{s}")
            for s in range(NS)
        ]
        QT = {}  # streamed, tags per chunk

        def emit_proj_stripe(s):
            """K^T, Q^T, V for the 512-token stripe s (shared x / cos / sin)."""
            sl = ds(s * 512, 512)
            for cp in range(4):
                xa = xpool.tile([128, 512], dt.bfloat16, tag="xa", name="xa")
                xb = xpool.tile([128, 512], dt.bfloat16, tag="xb", name="xb")
                nc.sync.dma_start(xa[:], xpT[cp, :, sl])
                nc.sync.dma_start(xb[:], xpT[cp + 4, :, sl])
                # reconstruct cos/sin for (chunk cp, stripe s) via angle addition
                cos = cspool.tile([128, 512], dt.bfloat16, tag="cos", name="cos")
                sin = cspool.tile([128, 512], dt.bfloat16, tag="sin", name="sin")
                t1 = cspool.tile([128, 512], dt.bfloat16, tag="t1", name="t1")
                t2 = cspool.tile([128, 512], dt.bfloat16, tag="t2", name="t2")
                chs = httab_sb[:, ds(cp * NS + s, 1)]
                shs = httab_sb[:, ds(32 + cp * NS + s, 1)]
                nc.vector.tensor_scalar_mul(t1[:], cosL_sb[cp], chs)
                nc.vector.tensor_scalar_mul(t2[:], sinL_sb[cp], shs)
                nc.vector.tensor_sub(cos[:], t1[:], t2[:])
                t3 = cspool.tile([128, 512], dt.bfloat16, tag="t1", name="t3")
                t4 = cspool.tile([128, 512], dt.bfloat16, tag="t2", name="t4")
                nc.vector.tensor_scalar_mul(t3[:], cosL_sb[cp], shs)
                nc.vector.tensor_scalar_mul(t4[:], sinL_sb[cp], chs)
                nc.vector.tensor_add(sin[:], t3[:], t4[:])

                def rope_pair(w_sb, out_e, out_o):
                    pe = psum.tile([128, 512], dt.float32, tag="A", name="A")
                    po = psum.tile([128, 512], dt.float32, tag="B", name="B")
                    nc.tensor.matmul(pe[:], lhsT=w_sb[cp], rhs=xa[:],
                                     start=True, stop=True)
                    nc.tensor.matmul(po[:], lhsT=w_sb[cp + 4], rhs=xb[:],
                                     start=True, stop=True)
                    ke = rtmp.tile([128, 512], dt.bfloat16, tag="ke", name="ke")
                    ko = rtmp.tile([128, 512], dt.bfloat16, tag="ko", name="ko")
                    nc.scalar.copy(ke[:], pe[:])
                    nc.scalar.copy(ko[:], po[:])
                    ta = rtmp.tile([128, 512], dt.bfloat16, tag="ta", name="ta")
                    tb = rtmp.tile([128, 512], dt.bfloat16, tag="tb", name="tb")
                    nc.vector.tensor_mul(ta[:], ke[:], cos[:])
                    nc.vector.tensor_mul(tb[:], ko[:], sin[:])
                    nc.vector.tensor_sub(out_e[:], ta[:], tb[:])
                    ta2 = rtmp.tile([128, 512], dt.bfloat16, tag="ta", name="ta")
                    tb2 = rtmp.tile([128, 512], dt.bfloat16, tag="tb", name="tb")
                    nc.vector.tensor_mul(ta2[:], ke[:], sin[:])
                    nc.vector.tensor_mul(tb2[:], ko[:], cos[:])
                    nc.vector.tensor_add(out_o[:], ta2[:], tb2[:])

                rope_pair(wk_sb, KT[(cp, s)], KT[(cp + 4, s)])
                QT[(cp, s)] = qpool.tile([128, 512], dt.bfloat16, tag=f"qt{cp}",
                                         name=f"qt{cp}")
                QT[(cp + 4, s)] = qpool.tile([128, 512], dt.bfloat16,
                                             tag=f"qt{cp + 4}", name=f"qt{cp + 4}")
                rope_pair(wq_sb, QT[(cp, s)], QT[(cp + 4, s)])

                va = psum1.tile([128, 4, 128], dt.float32, tag="YL", name="VA")
                vb = psum1.tile([128, 4, 128], dt.float32, tag="YH", name="VB")
                for sub in range(4):
                    nc.tensor.matmul(
                        va[:, sub, :], lhsT=xa[:, ts(sub, 128)], rhs=wv_sb[cp],
                        start=True, stop=True,
                    )
                    nc.tensor.matmul(
                        vb[:, sub, :], lhsT=xb[:, ts(sub, 128)], rhs=wv_sb[cp + 4],
                        start=True, stop=True,
                    )
                nc.any.tensor_copy(V[s][:, :, ds(cp * 128, 128)], va[:])
                nc.any.tensor_copy(V[s][:, :, ds((cp + 4) * 128, 128)], vb[:])

        def emit_q_tile(G):
            nblk = G + 1
            nst = (nblk + 3) // 4
            wlast = (nblk - 4 * (nst - 1)) * 128
            qs, qoff = G // 4, (G % 4) * 128
            y_lo = psum1.tile([128, 512], dt.float32, tag="YL", name="YL")
            y_hi = psum1.tile([128, 512], dt.float32, tag="YH", name="YH")
            l_parts = lpool.tile([128, NS], dt.float32, tag="lp", name="lp")
            for t in range(nst):
                w = 512 if t < nst - 1 else wlast
                S = psum.tile([128, 512], dt.float32, tag="A", name="A")
                for c in range(8):
                    nc.tensor.matmul(
                        S[:, :w],
                        lhsT=QT[(c, qs)][:, ds(qoff, 128)],
                        rhs=KT[(c, t)][:, :w],
                        start=(c == 0), stop=(c == 7),
                    )
                if t == nst - 1:
                    nc.vector.tensor_add(S[:, :w], S[:, :w], masks[G % 4][:, :w])
                P = ppool.tile([128, 512], dt.bfloat16, tag="p", name="p")
                nc.scalar.activation(
                    P[:, :w], S[:, :w], mybir.ActivationFunctionType.Exp,
                    scale=GAMMA, accum_out=l_parts[:, ds(t, 1)],
                )
                nb = w // 128
                pt_ps = psum.tile([128, 512], dt.bfloat16, tag="B", name="B")
                for b in range(nb):
                    nc.tensor.transpose(pt_ps[:, ts(b, 128)], P[:, ts(b, 128)],
                                        ident[:])
                pt = ptpool.tile([128, 512], dt.bfloat16, tag="pt", name="pt")
                nc.scalar.copy(pt[:, :w], pt_ps[:, :w])
                for b in range(nb):
                    blk = t * 4 + b
                    vs = V[blk // 4]
                    nc.tensor.matmul(y_lo[:], lhsT=pt[:, ts(b, 128)],
                                     rhs=vs[:, blk % 4, 0:512],
                                     start=(blk == 0), stop=(blk == nblk - 1))
                    nc.tensor.matmul(y_hi[:], lhsT=pt[:, ts(b, 128)],
                                     rhs=vs[:, blk % 4, 512:1024],
                                     start=(blk == 0), stop=(blk == nblk - 1))
            lsum = lpool.tile([128, 1], dt.float32, tag="ls", name="ls")
            linv = lpool.tile([128, 1], dt.float32, tag="li", name="li")
            nc.vector.tensor_reduce(lsum[:], l_parts[:, :nst],
                                    mybir.AxisListType.X, mybir.AluOpType.add)
            nc.vector.reciprocal(linv[:], lsum[:])
            y_sb = ypool.tile([128, 1024], dt.bfloat16, tag="y", name="y")
            nc.vector.tensor_scalar_mul(y_sb[:, 0:512], y_lo[:], linv[:])
            nc.vector.tensor_scalar_mul(y_sb[:, 512:1024], y_hi[:], linv[:])
            nc.sync.dma_start(y[ts(G, 128), :], y_sb[:])

        # Projection runs one stripe ahead of attention so its DMA + DVE
        # latency hides under the (PE-bound) attention of the prior stripe.
        emit_proj_stripe(0)
        for s in range(NS):
            if s + 1 < NS:
                emit_proj_stripe(s + 1)
            for G in range(4 * s, 4 * s + 4):
                emit_q_tile(G)

    nc.compile()
    return nc


# ------------------------- host side -------------------------


def pack_tables(w2q, w2k, w2v, tables):
    """wtab [128,7168] bf16 and httab [128,64] fp32 (shared by all cores)."""
    cosL_t, sinL_t, cosH_t, sinH_t = tables
    wtab = np.concatenate(
        [
            w2q.transpose(1, 0, 2).reshape(128, 1024),
            w2k.transpose(1, 0, 2).reshape(128, 1024),
            w2v.transpose(1, 0, 2).reshape(128, 1024),
            cosL_t.transpose(1, 0, 2).reshape(128, 2048),
            sinL_t.transpose(1, 0, 2).reshape(128, 2048),
        ],
        axis=1,
    ).astype(BF16)
    httab = np.concatenate(
        [
            cosH_t.transpose(1, 0, 2).reshape(128, 32),
            sinH_t.transpose(1, 0, 2).reshape(128, 32),
        ],
        axis=1,
    ).astype(np.float32)
    return wtab, httab


def prep_core_inputs(xb, wtab, httab):
    """Inputs for one core: batch slice xb (T, 1024) fp32."""
    xpT = np.ascontiguousarray(xb.T[PERM].reshape(8, 128, T)).astype(BF16)
    return {"xpT": xpT, "wtab": wtab, "httab": httab}


def core_model(inp):
    """Numpy model of one core's program (fp32 math, for tests)."""
    xpT = inp["xpT"].astype(np.float32)
    wtab = inp["wtab"].astype(np.float32)
    httab = inp["httab"].astype(np.float32)
    w2q = wtab[:, 0:1024].reshape(128, 8, 128).transpose(1, 0, 2)
    w2k = wtab[:, 1024:2048].reshape(128, 8, 128).transpose(1, 0, 2)
    w2v = wtab[:, 2048:3072].reshape(128, 8, 128).transpose(1, 0, 2)
    cosL_t = wtab[:, 3072:5120].reshape(128, 4, 512).transpose(1, 0, 2)
    sinL_t = wtab[:, 5120:7168].reshape(128, 4, 512).transpose(1, 0, 2)
    cosH_t = httab[:, 0:32].reshape(128, 4, 8).transpose(1, 0, 2)
    sinH_t = httab[:, 32:64].reshape(128, 4, 8).transpose(1, 0, 2)
    # reconstruct cos/sin as the device does
    cos = np.zeros((4, 128, T), dtype=np.float32)
    sin = np.zeros((4, 128, T), dtype=np.float32)
    for s in range(NS):
        ch = cosH_t[:, :, s:s + 1]
        sh = sinH_t[:, :, s:s + 1]
        cos[:, :, s * 512:(s + 1) * 512] = cosL_t * ch - sinL_t * sh
        sin[:, :, s * 512:(s + 1) * 512] = cosL_t * sh + sinL_t * ch

    def proj_T(w2):  # -> [8, 128, T]
        return np.stack([w2[c].T @ xpT[c] for c in range(8)])

    def rope(zT):
        out = np.empty_like(zT)
        for c in range(4):
            e, o = zT[c], zT[c + 4]
            out[c] = e * cos[c] - o * sin[c]
            out[c + 4] = e * sin[c] + o * cos[c]
        return out

    kT = rope(proj_T(w2k)).reshape(1024, T)
    qT = rope(proj_T(w2q)).reshape(1024, T)
    v = np.concatenate([w2v[c].T @ xpT[c] for c in range(8)], axis=0).T  # [T,1024]

    yout = np.zeros((T, 1024), dtype=np.float32)
    for G in range(NT):
        nblk = G + 1
        q = qT[:, G * 128:(G + 1) * 128].T  # [128, 1024]
        keys = kT[:, :nblk * 128]
        S = q @ keys
        k0 = 4 * ((nblk + 3) // 4 - 1) * 128
        r = G % 4
        pcol = np.arange(128)[:, None]
        ccol = np.arange(nblk * 128 - k0)[None, :]
        S[:, k0:] += np.where(ccol <= r * 128 + pcol, 0.0, -1e9)
        P = np.exp(GAMMA * S)
        yout[G * 128:(G + 1) * 128] = (P @ v[:nblk * 128]) / P.sum(1, keepdims=True)
    return yout


_NC_CACHE = {}
last_in_maps = None


def kernel(x, w_q, w_k, w_v):
    global last_in_maps
    from concourse.bass_utils import run_bass_kernel_spmd

    B, Tx, D = x.shape
    assert (B, Tx, D) == (4, 4096, 1024)
    x = np.asarray(x, dtype=np.float32)
    w2q = _block_weights(np.asarray(w_q, dtype=np.float32))
    w2k = _block_weights(np.asarray(w_k, dtype=np.float32))
    w2v = _block_weights(np.asarray(w_v, dtype=np.float32))
    tables = _rope_factor_tables()
    wtab, httab = pack_tables(w2q, w2k, w2v, tables)

    in_maps = [prep_core_inputs(x[b], wtab, httab) for b in range(4)]
    last_in_maps = in_maps

    if "nc" not in _NC_CACHE:
        _NC_CACHE["nc"] = build_nc()
    nc = _NC_CACHE["nc"]

    res = run_bass_kernel_spmd(nc, in_maps, core_ids=list(range(4)))
    out = np.zeros((B, Tx, D), dtype=np.float32)
    for b in range(4):
        out[b] = res.results[b]["y"].astype(np.float32)[:, INV_PERM]
    return out
